# revision 49
# baseline (speedup 1.0000x reference)
"""GCN2 (6-layer GCN2Conv) distributed Bass kernel for 8 TRN2 NeuronCores.

Strategy (dst-sharded message passing):
  - Destination nodes are sharded across the 8 cores via a DEGREE-BALANCED
    snake-deal permutation over the 8x49 (core, dst-block) cells: every
    block carries a near-equal edge count, which shrinks the SPMD tile
    padding (tiles per (core, half, block) = ceil over the max core) and
    keeps the ranks in lockstep at the AllGather barriers.
  - Per core the segment-sum is computed as PE matmuls over tiles of 128
    edges sorted by (src-half, dst-block): dma_gather pulls the 128 source
    rows (bf16, 256B descriptors) as the stationary lhsT [128e, 128f]; the
    one-hot scatter matrix [128e, 128d] ((iota==dst_local)*(1-alpha)*w) is
    precomputed on host and streamed from DRAM by the sync-engine HWDGE.
  - Gather descriptor generation (the Q7 SWDGE serial bottleneck,
    ~7ns/idx) is spread round-robin over all four SWDGE queues, each
    pinned to its own Q7 core pair, so four gathers' desc-gen pipelines
    concurrently (~2ns/idx, co-saturating the 16 SDMA engines).
  - The per-layer node-feature table is exchanged in two half-AllGathers
    (<=4095 rows/rank each: gather idxs are int16).  The A half's store +
    collective launch happen mid-tail-pass; the B half's trigger is
    deferred into the consumer layer's pass-A so the in-order Pool stream
    (which carries the gathers) never stalls on a collective's input DMA.
  - h rows are staged in SBUF and flushed to the exchange buffer with ONE
    rearranged-AP DMA per half (49 tiny HWDGE stores cost ~600ns of
    sequencer dispatch each and starve the collective trigger).
  - z accumulation is two-pass: pass A accumulates alpha*x0 (bf16 identity
    matmul) plus all A-half tiles into PSUM, evacuated to f32 SBUF zA by
    the DVE; pass B accumulates B-half tiles in PSUM and combines
    zT = psB + zA on the DVE, then the GCN2 tail:
    hT = relu((1-beta)zT + Wb^T zT), PE-transpose, staged for exchange.
"""

import math
import numpy as np
import ml_dtypes

import concourse.bass as bass
import concourse.bacc as bacc
import concourse.tile as tile
import concourse.mybir as mybir
from concourse.bass_utils import run_bass_kernel_spmd

# ---------------------------------------------------------------- constants
NCORES = 8
N = 50000
E = 800000
D = 128
OUT_DIM = 64
NCORE = N // NCORES          # 6250 nodes owned per core
NBLK = 49                    # dst blocks of 128 per core
NPAD = NBLK * 128            # 6272 padded nodes per core
# Half split of each rank's NPAD rows for the two staggered AllGathers.
# Constraint: 8*rows_per_rank <= 32767 per table (gather idx is int16),
# so each half must be <= 4095 rows/rank.
QBOUNDS = (0, 2816, NPAD)
NQ = len(QBOUNDS) - 1
QROWS = tuple(QBOUNDS[q + 1] - QBOUNDS[q] for q in range(NQ))
HALFA = QBOUNDS[1]
HALFB = NPAD - HALFA
ABLK = HALFA // 128          # blocks in half A (22)
NUM_CONVS = 6
ALPHA = 0.1
THETA = 0.5
LAYER_NUM = 8
BETA = math.log(THETA / (LAYER_NUM + 1) + 1.0)
CH = 16                      # gather chunk size in tiles (128 edges per tile)
# SWDGE queues used for dma_gather desc-gen.  Queue q runs on Q7 core pair
# (2q, 2q+1); gathers on distinct queues pipeline on distinct core pairs,
# parallelizing descriptor generation ~4x.  (The CC-core_* collective
# threads are a separate HW unit, not Q7 cores, so all 4 queues are free.)
GQUEUES = (0, 1, 2, 3)

F32 = mybir.dt.float32
BF16 = mybir.dt.bfloat16
I16 = mybir.dt.int16


# ------------------------------------------------------------ preprocessing
def build_perm(edge_dst):
    """Degree-balanced node -> global-slot permutation.

    Nodes are sorted by in-degree and snake-dealt across the 8*NBLK
    (core, block) cells so every cell carries a near-equal edge count.
    This shrinks the per-(core, half, block) tile-count max (ceil over the
    max core) that pads the gather index stream, and evens the per-core
    load so ranks arrive at the AllGather barriers together.

    Returns perm[n] = global slot (core = slot // NPAD, local = slot % NPAD).
    """
    deg = np.bincount(np.asarray(edge_dst).astype(np.int64), minlength=N)
    order = np.argsort(-deg, kind="stable")          # high degree first
    ncells = NCORES * NBLK
    perm = np.empty(N, np.int64)
    for r in range(0, N, ncells):                    # deal one round per cell
        chunk = order[r:r + ncells]
        cells = np.arange(chunk.shape[0])
        if (r // ncells) & 1:
            cells = ncells - 1 - cells               # snake direction
        slot = r // ncells
        core = cells // NBLK
        blk = cells % NBLK
        perm[chunk] = core * NPAD + blk * 128 + slot
    return perm


def preprocess(edge_src, edge_dst, edge_weight, perm=None):
    """Sort/pad edges per (core, src-half, dst-block); build device arrays.

    Returns (structure, arrays):
      structure: tuple of NQ tuples, tiles per dst-block per half
        (shared by all cores so the SPMD program is identical).
      arrays: idx{q} [8,128,S_q*8] i16 (dma_gather layout: idx j at
        partition j%16 col j//16, replicated 8x over partitions),
        onehot [8,128,Ttot,128] bf16 (partition-major one-hot tiles:
        [c, slot, tile, dstcol] = (1-alpha)*w for the edge at that slot).
    """
    es = np.asarray(edge_src).astype(np.int64)
    ed = np.asarray(edge_dst).astype(np.int64)
    ew = np.asarray(edge_weight).astype(np.float32)
    ne = es.shape[0]

    if perm is None:           # identity layout: range-sharded nodes
        n_ = np.arange(N, dtype=np.int64)
        perm = (n_ // NCORE) * NPAD + (n_ % NCORE)
    pd = perm[ed]
    core = pd // NPAD
    dl = pd % NPAD
    blk = dl >> 7
    dloc = dl & 127
    ps = perm[es]
    s_core = ps // NPAD
    s_loc = ps % NPAD
    qtr = np.searchsorted(np.asarray(QBOUNDS[1:-1]), s_loc, side="right")
    srcq = (s_core * np.asarray(QROWS)[qtr]
            + (s_loc - np.asarray(QBOUNDS[:-1])[qtr]))

    key = (qtr * NBLK + blk) * NCORES + core  # q-major, then block, then core
    counts = np.bincount(key, minlength=NQ * NBLK * NCORES) \
        .reshape(NQ, NBLK, NCORES)
    tiles_qb = np.maximum(1, -(-counts.max(axis=2) // 128))    # [NQ, NBLK]
    T = [tiles_qb[q].astype(int) for q in range(NQ)]
    S = [int(t.sum()) for t in T]
    O = [np.concatenate([[0], np.cumsum(t)[:-1]]).astype(int) for t in T]
    CS = np.concatenate([[0], np.cumsum(S)[:-1]]).astype(int)  # col offsets
    Ttot = sum(S)

    skey = (core * NQ + qtr) * NBLK + blk     # rank within (c, q, b) group
    order = np.argsort(skey, kind="stable")
    ks = skey[order]
    grp_start = np.searchsorted(ks, np.arange(NCORES * NQ * NBLK))
    r = np.arange(ne) - grp_start[ks]

    c_s = core[order]
    b_s = blk[order]
    q_s = qtr[order]
    sp = srcq[order]
    dlo = dloc[order]
    wv = ew[order]

    Oq = np.stack([O[q] for q in range(NQ)], axis=0)      # [NQ, NBLK]
    pos = Oq[q_s, b_s] * 128 + r                  # slot in half stream

    idxs = [np.zeros((NCORES, S[q] * 128), np.int16) for q in range(NQ)]
    onehot = np.zeros((NCORES, 128, Ttot, 128), ml_dtypes.bfloat16)

    for q in range(NQ):
        m = q_s == q
        idxs[q][c_s[m], pos[m]] = sp[m].astype(np.int16)
        t = CS[q] + (pos[m] >> 7)
        p = pos[m] & 127
        onehot[c_s[m], p, t, dlo[m]] = ((1.0 - ALPHA) * wv[m]).astype(
            ml_dtypes.bfloat16)

    def pack_idx(idx, Sq):
        a = idx.reshape(NCORES, Sq * 8, 16).transpose(0, 2, 1)  # [8,16,S*8]
        return np.ascontiguousarray(np.tile(a, (1, 8, 1)))      # [8,128,S*8]

    arrays = {f"idx{q}": pack_idx(idxs[q], S[q]) for q in range(NQ)}
    arrays["onehot"] = onehot
    structure = tuple(tuple(t.tolist()) for t in T)
    return structure, arrays


# ----------------------------------------------------------------- builder
def build(structure, num_convs=NUM_CONVS):
    T = [list(t) for t in structure]
    S = [sum(t) for t in T]
    O = [np.concatenate([[0], np.cumsum(t)[:-1]]).astype(int) for t in T]
    CS = np.concatenate([[0], np.cumsum(S)[:-1]]).astype(int)
    Ttot = sum(S)

    nc = bacc.Bacc("TRN2", target_bir_lowering=False, debug=False,
                   num_devices=NCORES, num_swdge_queues=4)

    xT_d = nc.dram_tensor("xT", [D, NPAD], BF16, kind="ExternalInput")
    idx_d = [nc.dram_tensor(f"idx{q}", [128, S[q] * 8], I16,
                            kind="ExternalInput") for q in range(NQ)]
    oh_d = nc.dram_tensor("onehot", [128, Ttot, 128], BF16,
                          kind="ExternalInput")
    w0_d = nc.dram_tensor("W0", [D, D], BF16, kind="ExternalInput")
    wb_d = nc.dram_tensor("Wb", [NUM_CONVS, D, D], F32, kind="ExternalInput")
    w1_d = nc.dram_tensor("W1", [D, OUT_DIM], F32, kind="ExternalInput")
    b0_d = nc.dram_tensor("b0c", [D, 1], F32, kind="ExternalInput")
    b1_d = nc.dram_tensor("b1c", [OUT_DIM, 1], F32, kind="ExternalInput")
    aI_d = nc.dram_tensor("alphaI", [D, D], BF16, kind="ExternalInput")
    oI_d = nc.dram_tensor("ombI", [D, D], F32, kind="ExternalInput")
    id_d = nc.dram_tensor("identT", [D, D], F32, kind="ExternalInput")
    idb_d = nc.dram_tensor("identB", [D, D], BF16, kind="ExternalInput")
    out_d = nc.dram_tensor("outT", [NBLK, OUT_DIM, 128], F32,
                           kind="ExternalOutput")

    ts = mybir.AluOpType
    AF = mybir.ActivationFunctionType

    with tile.TileContext(nc) as tc:
        from contextlib import ExitStack
        with ExitStack() as ctx:
            rp = ctx.enter_context(tc.tile_pool(name="resident", bufs=1))
            # The effective gather pipeline depth is min(gp, op): the tile
            # matmuls consume a gather tile and its onehot tile together,
            # so each pool's slot reuse gates its stream (measured: 20/8
            # regressed to depth-8 behavior; 14/14 is the SBUF-feasible max).
            gp = ctx.enter_context(tc.tile_pool(name="gp", bufs=14))
            op = ctx.enter_context(tc.tile_pool(name="op", bufs=14))
            ztp = ctx.enter_context(tc.tile_pool(name="ztp", bufs=3))
            htp = ctx.enter_context(tc.tile_pool(name="htp", bufs=3))
            stp = ctx.enter_context(tc.tile_pool(name="stp", bufs=1))
            obp = ctx.enter_context(tc.tile_pool(name="obp", bufs=2))
            pzp = ctx.enter_context(
                tc.tile_pool(name="pzp", bufs=4, space="PSUM"))
            php = ctx.enter_context(
                tc.tile_pool(name="php", bufs=2, space="PSUM"))
            pap = ctx.enter_context(
                tc.tile_pool(name="pap", bufs=2, space="PSUM"))
            drp = ctx.enter_context(
                tc.tile_pool(name="drp", bufs=1, space="DRAM"))

            # ---------------- resident tiles
            idx_sb = [rp.tile([128, S[q] * 8], I16, name=f"idx{q}sb",
                              tag=f"idx{q}sb") for q in range(NQ)]
            w0_sb = rp.tile([D, D], BF16, name="w0sb", tag="w0sb")
            wb_sb = rp.tile([D, NUM_CONVS * D], F32, name="wbsb", tag="wbsb")
            w1_sb = rp.tile([D, OUT_DIM], F32, name="w1sb", tag="w1sb")
            b0_sb = rp.tile([D, 1], F32, name="b0sb", tag="b0sb")
            b1_sb = rp.tile([OUT_DIM, 1], F32, name="b1sb", tag="b1sb")
            aI_sb = rp.tile([D, D], BF16, name="aIsb", tag="aIsb")
            oI_sb = rp.tile([D, D], F32, name="oIsb", tag="oIsb")
            id_sb = rp.tile([D, D], F32, name="idsb", tag="idsb")
            idb_sb = rp.tile([D, D], BF16, name="idbsb", tag="idbsb")
            x0T_sb = rp.tile([D, NPAD], BF16, name="x0Tsb", tag="x0Tsb")
            x_sb = rp.tile([D, NPAD], BF16, name="xsb", tag="xsb")
            zA_sb = rp.tile([D, NPAD], F32, name="zAsb", tag="zAsb")

            nc.sync.dma_start(x_sb[:], xT_d[:])
            for q in range(NQ):
                nc.sync.dma_start(idx_sb[q][:], idx_d[q][:])
            nc.sync.dma_start(w0_sb[:], w0_d[:])
            for i in range(NUM_CONVS):
                nc.sync.dma_start(wb_sb[:, i * D:(i + 1) * D], wb_d[i, :, :])
            nc.sync.dma_start(w1_sb[:], w1_d[:])
            nc.sync.dma_start(b0_sb[:], b0_d[:])
            nc.sync.dma_start(b1_sb[:], b1_d[:])
            nc.sync.dma_start(aI_sb[:], aI_d[:])
            nc.sync.dma_start(oI_sb[:], oI_d[:])
            nc.sync.dma_start(id_sb[:], id_d[:])
            nc.sync.dma_start(idb_sb[:], idb_d[:])

            # exchange buffers: per layer, per half
            ag_in = [drp.tile([NPAD, D], BF16, name=f"agin{k}",
                              tag=f"agin{k}") for k in range(num_convs)]
            ag_out = [[drp.tile([QROWS[q] * NCORES, D], BF16,
                                name=f"agout{k}q{q}", tag=f"agout{k}q{q}",
                                addr_space="Shared") for q in range(NQ)]
                      for k in range(num_convs)]

            rg = [list(range(NCORES))]

            def launch_ag(k, q):
                # The trigger lives in the Pool stream; its wait on the
                # ag_in store stalls every gather queued behind it, so the
                # call sites place it where the store is already complete
                # (data-ready-aligned), and the deep gather buffering
                # (bufs=12) keeps the Q7 pairs busy through residual waits.
                lo = QBOUNDS[q]
                hi = QBOUNDS[q + 1]
                nc.gpsimd.collective_compute(
                    "AllGather", ts.bypass, replica_groups=rg,
                    ins=[ag_in[k][lo:hi, :].opt()],
                    outs=[ag_out[k][q][:].opt()])

            # gather chunk boundaries per half
            CB = []
            for q in range(NQ):
                bnd = list(range(0, S[q], CH)) + [S[q]]
                CB.append(bnd)

            # Per-layer SBUF staging of the h rows, flushed with ONE batched
            # DMA per half: 49 tiny per-block HWDGE stores cost ~600ns of
            # sequencer dispatch each and the AllGather trigger ends up
            # waiting on the last of them (the sequencer streams are the
            # hidden bottleneck once gather desc-gen is parallelized).
            stages = {}

            def stage_for(k):
                if k not in stages:
                    stages[k] = stp.tile([128, NPAD], BF16, name="stg",
                                         tag="stg")
                return stages[k]

            def store_block(hT, b, k):
                """Transpose h^T block -> h rows into the staging tile."""
                pt = pap.tile([128, 128], F32, name="pt", tag="paux")
                nc.tensor.transpose(pt[:], hT, id_sb[:])
                st = stage_for(k)
                nc.scalar.activation(st[:, b * 128:(b + 1) * 128], pt[:],
                                     AF.Copy)

            def store_half(k, q):
                """Flush one half of the staging tile to ag_in (DMA only)."""
                st = stage_for(k)
                lo = QBOUNDS[q]
                hi = QBOUNDS[q + 1]
                nc.scalar.dma_start(
                    ag_in[k][lo:hi, :].rearrange("(b p) c -> p b c", p=128),
                    st[:, lo:hi])

            # ---------------- layer 0: x0^T = relu(W0^T x^T + b0)
            for b in range(NBLK):
                ps = pzp.tile([128, 128], F32, name="ps0", tag="pz")
                nc.tensor.matmul(ps[:], w0_sb[:],
                                 x_sb[:, b * 128:(b + 1) * 128],
                                 start=True, stop=True)
                nc.scalar.activation(x0T_sb[:, b * 128:(b + 1) * 128],
                                     ps[:], AF.Relu, bias=b0_sb[:])
                pt0 = pap.tile([128, 128], BF16, name="pt0", tag="paux")
                nc.tensor.transpose(
                    pt0[:], x0T_sb[:, b * 128:(b + 1) * 128], idb_sb[:])
                st0 = stage_for(0)
                # DVE, not scalar: layer 0's path to the first AllGather is
                # paced by the scalar stream (relu+copy per block); the
                # vector engine is idle here and halves that chain.
                nc.vector.tensor_scalar(
                    st0[:, b * 128:(b + 1) * 128], pt0[:], 0.0, None, ts.add)
                if b == ABLK - 1:
                    store_half(0, 0)
                    launch_ag(0, 0)
            store_half(0, 1)
            launch_ag(0, 1)

            # ---------------- conv layers
            PREFETCH = 9
            gq_counter = [0]
            for i in range(num_convs):
                gbufs = {}

                def g_emit(q, k, i=i, gbufs=gbufs):
                    lo = int(CB[q][k])
                    hi = int(CB[q][k + 1])
                    nt = hi - lo
                    gt = gp.tile([128, nt, 128], BF16, name="gt", tag="gt")
                    qn = GQUEUES[gq_counter[0] % len(GQUEUES)]
                    gq_counter[0] += 1
                    nc.gpsimd.dma_gather(
                        gt[:], ag_out[i][q][:], idx_sb[q][:, lo * 8:hi * 8],
                        nt * 128, nt * 128, D, single_packet=False,
                        queue_num=qn)
                    ot = op.tile([128, nt, 128], BF16, name="ot", tag="ot")
                    nc.sync.dma_start(
                        ot[:], oh_d[:, CS[q] + lo:CS[q] + hi, :])
                    gbufs[(q, k)] = (gt, ot, lo)

                def g_ap(q, s, gbufs=gbufs):
                    k = int(np.searchsorted(CB[q], s, side="right")) - 1
                    gt, ot, lo = gbufs[(q, k)]
                    return gt[:, s - lo, :], ot[:, s - lo, :]

                emitted = [0, 0]

                def need_chunks(q, b, emitted=emitted):
                    last = O[q][b] + T[q][b] - 1
                    k = int(np.searchsorted(CB[q], last, side="right")) - 1
                    top = min(k + PREFETCH, len(CB[q]) - 2)
                    while emitted[q] <= top:
                        g_emit(q, emitted[q])
                        emitted[q] += 1

                # pass A: psum = alpha*x0 + sum(A-half tiles); evac to zA
                for b in range(NBLK):
                    need_chunks(0, b)
                    if i >= 1 and b == 13:
                        # B-half AllGather of THIS layer: trigger here so
                        # the Pool stream reaches it right as the previous
                        # layer's tail stores complete (no mid-stream stall).
                        launch_ag(i, 1)
                    ps = pzp.tile([128, 128], F32, name="psa", tag="pz")
                    nc.tensor.matmul(ps[:], aI_sb[:],
                                     x0T_sb[:, b * 128:(b + 1) * 128],
                                     start=True, stop=False)
                    for j in range(T[0][b]):
                        g, oh = g_ap(0, O[0][b] + j)
                        nc.tensor.matmul(ps[:], g, oh, start=False,
                                         stop=(j == T[0][b] - 1))
                    nc.vector.tensor_scalar(
                        zA_sb[:, b * 128:(b + 1) * 128], ps[:], 0.0, None,
                        ts.add)

                # pass B: psum = sum(B-half tiles); zT = psB + zA; tail
                for b in range(NBLK):
                    need_chunks(1, b)
                    ps = pzp.tile([128, 128], F32, name="psb", tag="pz")
                    for j in range(T[1][b]):
                        g, oh = g_ap(1, O[1][b] + j)
                        nc.tensor.matmul(ps[:], g, oh, start=(j == 0),
                                         stop=(j == T[1][b] - 1))
                    zT = ztp.tile([128, 128], F32, name="zT", tag="zT")
                    nc.vector.scalar_tensor_tensor(
                        zT[:], ps[:], 0.0,
                        zA_sb[:, b * 128:(b + 1) * 128], ts.add, ts.add)
                    ph = php.tile([128, 128], F32, name="ph", tag="ph")
                    nc.tensor.matmul(ph[:], wb_sb[:, i * D:(i + 1) * D],
                                     zT[:], start=True, stop=False)
                    nc.tensor.matmul(ph[:], oI_sb[:], zT[:],
                                     start=False, stop=True)
                    hT = htp.tile([128, 128], F32, name="hT", tag="hT")
                    nc.scalar.activation(hT[:], ph[:], AF.Relu)
                    if i < num_convs - 1:
                        store_block(hT[:], b, i + 1)
                        if b == ABLK - 1:
                            store_half(i + 1, 0)
                        if b == 31:
                            launch_ag(i + 1, 0)
                    else:
                        po = pap.tile([OUT_DIM, 128], F32, name="po",
                                      tag="paux")
                        nc.tensor.matmul(po[:], w1_sb[:], hT[:],
                                         start=True, stop=True)
                        ob = obp.tile([OUT_DIM, 128], F32, name="ob",
                                      tag="ob")
                        nc.vector.tensor_scalar(
                            ob[:], po[:], b1_sb[:], None, ts.add)
                        nc.sync.dma_start(out_d[b, :, :], ob[:])

                # B-half store flushes here; its AllGather trigger is
                # deferred into the NEXT layer's pass-A (b==13) so the Pool
                # stream doesn't stall on the tail stores' completion.
                if i < num_convs - 1:
                    store_half(i + 1, 1)

    nc.compile()
    return nc


# ------------------------------------------------------------- host driver
def make_in_maps(x, W0, b0, W1, b1, conv_ws, arrays, perm):
    x = np.asarray(x, np.float32)
    flat = np.zeros((NCORES * NPAD, D), np.float32)
    flat[perm] = x
    xTp = np.ascontiguousarray(
        flat.reshape(NCORES, NPAD, D).transpose(0, 2, 1))
    ident = np.eye(D, dtype=np.float32)
    common = dict(
        W0=np.ascontiguousarray(
            np.asarray(W0, np.float32).astype(ml_dtypes.bfloat16)),
        Wb=np.ascontiguousarray(BETA * np.asarray(conv_ws, np.float32)),
        W1=np.ascontiguousarray(np.asarray(W1, np.float32)),
        b0c=np.ascontiguousarray(np.asarray(b0, np.float32).reshape(D, 1)),
        b1c=np.ascontiguousarray(
            np.asarray(b1, np.float32).reshape(OUT_DIM, 1)),
        alphaI=np.ascontiguousarray(
            (ALPHA * ident).astype(ml_dtypes.bfloat16)),
        ombI=np.ascontiguousarray((1.0 - BETA) * ident),
        identT=ident,
        identB=np.ascontiguousarray(ident.astype(ml_dtypes.bfloat16)),
    )
    in_maps = []
    for c in range(NCORES):
        m = dict(common)
        m["xT"] = np.ascontiguousarray(xTp[c].astype(ml_dtypes.bfloat16))
        for q in range(NQ):
            m[f"idx{q}"] = np.ascontiguousarray(arrays[f"idx{q}"][c])
        m["onehot"] = np.ascontiguousarray(arrays["onehot"][c])
        in_maps.append(m)
    return in_maps


def assemble_output(results, perm):
    rows = np.concatenate(
        [results[c]["outT"].transpose(0, 2, 1).reshape(NPAD, OUT_DIM)
         for c in range(NCORES)], axis=0)             # [8*NPAD, 64]
    return np.ascontiguousarray(rows[perm])


_CACHE = {}


def kernel(x, edge_src, edge_dst, edge_weight, W0, b0, W1, b1, conv_ws,
           _trace=False, _trace_kwargs=None):
    perm = build_perm(edge_dst)
    structure, arrays = preprocess(edge_src, edge_dst, edge_weight, perm)
    if structure not in _CACHE:
        _CACHE.clear()
        _CACHE[structure] = build(structure)
    nc = _CACHE[structure]
    in_maps = make_in_maps(x, W0, b0, W1, b1, conv_ws, arrays, perm)
    res = run_bass_kernel_spmd(
        nc, in_maps, core_ids=list(range(NCORES)), trace=_trace,
        **(_trace_kwargs or {}))
    out = assemble_output(res.results, perm)
    kernel.last_results = res
    return out

